# revision 11
# baseline (speedup 1.0000x reference)
"""Trainium2 Bass kernel for ChameleonVQVAEEncoderAttnBlock.

Reference computation (per batch b of 16, C=512 channels, N=32*32=1024 spatial):
    h  = GroupNorm32(x) * gamma + beta
    q, k, v = wq@h+bq, wk@h+bk, wv@h+bv          (1x1 convs == channel matmuls)
    S[i,j] = sum_c q[c,i] k[c,j] / sqrt(C)
    A = softmax_j(S)
    o[c,i] = sum_j v[c,j] A[i,j]
    y = wo@o + bo + x

Sharding: pure data parallel, batch 16 -> 2 batches on each of 8 cores.

Per-core kernel strategy (all matmuls in fp32r: 11-bit-mantissa fp32, full
PE rate at N=512):
  - S is computed TRANSPOSED (j on partitions, i free) so the softmax sum
    over j becomes a ones-vector matmul and A feeds the second matmul with
    no transposes at all.  Softmax max-subtraction is skipped (S ~ N(0,1),
    exp never overflows in fp32).
  - v is computed transposed (vT[s, c]) directly by the projection matmul
    (h slice as the stationary operand).
  - exp(S^T) is left unnormalized; the 1/sum factor (per column i) is
    applied to the attention output via a partition-broadcast multiply.
  - bv folds into an effective output bias bo_eff = bo + wo@bv (host).
  - The residual is added via DRAM: y <- x (DMA copy), then y += wo@o + bo
    via gpsimd accumulate-DMA.
"""
import numpy as np

import concourse.bacc as bacc
import concourse.mybir as mybir
import concourse.tile as tile
from concourse import bass_utils
from concourse.tile_rust import add_dep_helper

F32 = mybir.dt.float32
F32R = mybir.dt.float32r
AF = mybir.ActivationFunctionType
ALU = mybir.AluOpType

B, C, HH, WW = 16, 512, 32, 32
N = HH * WW          # 1024 spatial positions
NCORES = 8
NB = B // NCORES     # batches per core
CCH = C // 128       # 4 channel chunks
SCH = N // 128       # 8 spatial chunks
NIH = N // 512       # 2 free-dim halves
GROUPS = 32
GPC = C // GROUPS    # 16 channels per group
EPS = 1e-6
SCALE = float(C) ** -0.5


def _build_program():
    nc = bacc.Bacc("TRN2", target_bir_lowering=False, debug=False)

    x_d = nc.dram_tensor("x", [NB, C, N], F32, kind="ExternalInput").ap()
    w_d = {
        name: nc.dram_tensor(name, [C, C], F32R, kind="ExternalInput").ap()
        for name in ("wqT", "wkT", "wvT", "woT")
    }
    bq_d = nc.dram_tensor("bq", [C], F32, kind="ExternalInput").ap()
    bk_d = nc.dram_tensor("bk", [C], F32, kind="ExternalInput").ap()
    bo_d = nc.dram_tensor("bo", [C], F32, kind="ExternalInput").ap()
    gamma_d = nc.dram_tensor("gamma", [C], F32, kind="ExternalInput").ap()
    wsel_d = nc.dram_tensor("wsel", [128, 128], F32, kind="ExternalInput").ap()
    beta_d = nc.dram_tensor("beta", [C], F32, kind="ExternalInput").ap()
    y_d = nc.dram_tensor("y", [NB, C, N], F32, kind="ExternalOutput").ap()

    with tile.TileContext(nc) as tc:
        with (
            tc.tile_pool(name="const", bufs=1) as cp,
            tc.tile_pool(name="data", bufs=1) as dp,
            tc.tile_pool(name="psum", bufs=4, space="PSUM") as pp,
        ):
            # ---- constants / weights ------------------------------------
            wts = {}
            for name in ("wqT", "wkT", "wvT", "woT"):
                for ci in range(CCH):
                    t = cp.tile([128, C], F32R, name=f"{name}_{ci}")
                    nc.sync.dma_start(out=t, in_=w_d[name][ci * 128:(ci + 1) * 128, :])
                    wts[(name, ci)] = t

            def per_chunk_vec(name, src):
                out = []
                for cc in range(CCH):
                    t = cp.tile([128, 1], F32, name=f"{name}_{cc}")
                    nc.sync.dma_start(out=t, in_=src[cc * 128:(cc + 1) * 128])
                    out.append(t)
                return out

            bq_pl = per_chunk_vec("bqv", bq_d)
            bk_pl = per_chunk_vec("bkv", bk_d)
            bo_pl = per_chunk_vec("bov", bo_d)
            gamma_pl = per_chunk_vec("gammav", gamma_d)
            beta_pl = per_chunk_vec("betav", beta_d)

            eps_t = cp.tile([128, 1], F32)
            nc.vector.memset(eps_t, EPS)
            ones_t = cp.tile([128, 1], F32R)
            nc.scalar.activation(
                ones_t, nc.const_aps.tensor(1.0, (128, 1), F32), AF.Identity
            )
            # block-diagonal group-mean matrix: wsel[p', p] = 1/16 if same group
            wsel = cp.tile([128, 128], F32)
            nc.sync.dma_start(out=wsel, in_=wsel_d)

            # ---- per-batch tiles (tags rotate slots across batches) -----
            def batch_tiles(mk):
                return [mk(b) for b in range(NB)]

            x_t = batch_tiles(lambda b: [
                dp.tile([128, N], F32, name=f"x{cc}_b{b}", tag=f"x{cc}", bufs=2)
                for cc in range(CCH)
            ])
            # h shares slots with the attention output (tag h{cc})
            h_t = batch_tiles(lambda b: [
                dp.tile([128, N], F32R, name=f"h{cc}_b{b}", tag=f"h{cc}", bufs=2)
                for cc in range(CCH)
            ])

            # ---- load x + residual base copy + groupnorm (both batches) --
            copy_insts = {}
            for b in range(NB):
                for cc in range(CCH):
                    nc.sync.dma_start(
                        out=x_t[b][cc], in_=x_d[b, cc * 128:(cc + 1) * 128, :]
                    )
                    # residual base: y <- x  (DRAM -> DRAM)
                    copy_insts[(b, cc)] = nc.sync.dma_start(
                        out=y_d[b, cc * 128:(cc + 1) * 128, :],
                        in_=x_d[b, cc * 128:(cc + 1) * 128, :],
                    )

            for b in range(NB):
                for cc in range(CCH):
                    xc = x_t[b][cc]
                    stats6 = dp.tile([128, 2, 6], F32, name=f"st6_{b}_{cc}",
                                     tag="st6", bufs=2)
                    nc.vector.bn_stats(out=stats6[:, 0, :], in_=xc[:, 0:512])
                    nc.vector.bn_stats(out=stats6[:, 1, :], in_=xc[:, 512:N])
                    mv = dp.tile([128, 2], F32, name=f"mv_{b}_{cc}", tag="mv", bufs=2)
                    nc.vector.bn_aggr(out=mv, in_=stats6)
                    stacked = dp.tile([128, 2], F32, name=f"stk_{b}_{cc}",
                                      tag="stk", bufs=2)
                    nc.vector.tensor_copy(out=stacked[:, 0:1], in_=mv[:, 0:1])
                    nc.vector.tensor_mul(stacked[:, 1:2], mv[:, 0:1], mv[:, 0:1])
                    nc.vector.tensor_add(stacked[:, 1:2], stacked[:, 1:2], mv[:, 1:2])
                    # per-channel group stats: psg[p] = (mean_g, E[x^2]_g)
                    psg = pp.tile([128, 2], F32, name=f"psg_{b}_{cc}",
                                  tag="stat", bufs=2)
                    nc.tensor.matmul(psg, wsel, stacked, start=True, stop=True)
                    g2 = dp.tile([128, 2], F32, name=f"g2_{b}_{cc}", tag="g2", bufs=2)
                    nc.vector.tensor_copy(out=g2, in_=psg)
                    msq = dp.tile([128, 1], F32, name=f"msq_{b}_{cc}", tag="msq", bufs=2)
                    nc.vector.tensor_mul(msq, g2[:, 0:1], g2[:, 0:1])
                    var = dp.tile([128, 1], F32, name=f"var_{b}_{cc}", tag="var", bufs=2)
                    nc.vector.tensor_sub(var, g2[:, 1:2], msq)
                    std = dp.tile([128, 1], F32, name=f"std_{b}_{cc}", tag="std", bufs=2)
                    nc.scalar.activation(std, var, AF.Sqrt, bias=eps_t)
                    rstd = dp.tile([128, 1], F32, name=f"rstd_{b}_{cc}",
                                   tag="rstd", bufs=2)
                    nc.vector.reciprocal(rstd, std)
                    acoef = dp.tile([128, 1], F32, name=f"ac_{b}_{cc}", tag="ac", bufs=2)
                    nc.vector.tensor_mul(acoef, rstd, gamma_pl[cc])
                    bcoef = dp.tile([128, 1], F32, name=f"bc_{b}_{cc}", tag="bc", bufs=2)
                    nc.vector.tensor_mul(bcoef, g2[:, 0:1], acoef)
                    nc.vector.tensor_sub(bcoef, beta_pl[cc], bcoef)
                    # h = x * A + B   (rounded to f32r on write)
                    nc.vector.tensor_scalar(
                        out=h_t[b][cc], in0=xc, scalar1=acoef, scalar2=bcoef,
                        op0=ALU.mult, op1=ALU.add,
                    )

            # ---- attention per batch ------------------------------------
            for b in range(NB):
                h = h_t[b]
                q_t = [dp.tile([128, N], F32R, name=f"q{cc}_b{b}", tag=f"q{cc}")
                       for cc in range(CCH)]
                k_t = [dp.tile([128, N], F32R, name=f"k{cc}_b{b}", tag=f"k{cc}")
                       for cc in range(CCH)]
                vt_t = [dp.tile([128, C], F32R, name=f"vt{sc}_b{b}", tag=f"vt{sc}")
                        for sc in range(SCH)]

                # Q and K projections: q[co, s] = sum_ci wqT[ci, co]^T h[ci, s]
                for name, out_t, bias in (("wqT", q_t, bq_pl), ("wkT", k_t, bk_pl)):
                    for co in range(CCH):
                        for ih in range(NIH):
                            ps = pp.tile([128, 512], F32, name=f"ps_{name}{co}{ih}_b{b}",
                                         tag="mm")
                            for ci in range(CCH):
                                nc.tensor.matmul(
                                    ps,
                                    wts[(name, ci)][:, co * 128:(co + 1) * 128],
                                    h[ci][:, ih * 512:(ih + 1) * 512],
                                    start=(ci == 0), stop=(ci == CCH - 1),
                                )
                            nc.vector.tensor_scalar_add(
                                out=out_t[co][:, ih * 512:(ih + 1) * 512],
                                in0=ps, scalar1=bias[co],
                            )

                # V^T projection: vT[s, c] = sum_ci h[ci, s]^T wvT[ci, c]
                for sc in range(SCH):
                    ps = pp.tile([128, 512], F32, name=f"ps_vt{sc}_b{b}", tag="mm")
                    for ci in range(CCH):
                        nc.tensor.matmul(
                            ps,
                            h[ci][:, sc * 128:(sc + 1) * 128],
                            wts[("wvT", ci)],
                            start=(ci == 0), stop=(ci == CCH - 1),
                        )
                    nc.vector.tensor_copy(out=vt_t[sc], in_=ps)

                # S^T = K^T Q (j on partitions), then E = exp(S^T * scale)
                e_t = [dp.tile([128, N], F32R, name=f"E{jc}_b{b}", tag=f"E{jc}")
                       for jc in range(SCH)]
                for jc in range(SCH):
                    for ih in range(NIH):
                        ps = pp.tile([128, 512], F32, name=f"ps_s{jc}{ih}_b{b}",
                                     tag="mm")
                        for ci in range(CCH):
                            nc.tensor.matmul(
                                ps,
                                k_t[ci][:, jc * 128:(jc + 1) * 128],
                                q_t[ci][:, ih * 512:(ih + 1) * 512],
                                start=(ci == 0), stop=(ci == CCH - 1),
                            )
                        nc.scalar.activation(
                            out=e_t[jc][:, ih * 512:(ih + 1) * 512],
                            in_=ps, func=AF.Exp, scale=SCALE,
                        )

                # denom[i] = sum_j E[j, i]; rdb = broadcast(1/denom)
                rd = dp.tile([1, N], F32, name=f"rd_b{b}", tag="rd", bufs=1)
                for ih in range(NIH):
                    psd = pp.tile([1, 512], F32, name=f"psd{ih}_b{b}",
                                  tag="den", bufs=2)
                    for jc in range(SCH):
                        nc.tensor.matmul(
                            psd, ones_t, e_t[jc][:, ih * 512:(ih + 1) * 512],
                            start=(jc == 0), stop=(jc == SCH - 1),
                        )
                    nc.vector.reciprocal(rd[:, ih * 512:(ih + 1) * 512], psd)
                rdb = dp.tile([128, N], F32, name=f"rdb_b{b}", tag="rdb", bufs=1)
                nc.gpsimd.partition_broadcast(rdb, rd)

                # attention output o[c, i] = sum_j vT[j, c] E[j, i], normalized.
                # on_n shares slots with h (tag h{cc}).
                on_t = [dp.tile([128, N], F32R, name=f"on{cc}_b{b}", tag=f"h{cc}",
                                bufs=2) for cc in range(CCH)]
                for cc in range(CCH):
                    for ih in range(NIH):
                        ps = pp.tile([128, 512], F32, name=f"ps_av{cc}{ih}_b{b}",
                                     tag="mm")
                        for jc in range(SCH):
                            nc.tensor.matmul(
                                ps,
                                vt_t[jc][:, cc * 128:(cc + 1) * 128],
                                e_t[jc][:, ih * 512:(ih + 1) * 512],
                                start=(jc == 0), stop=(jc == SCH - 1),
                            )
                        nc.vector.tensor_mul(
                            on_t[cc][:, ih * 512:(ih + 1) * 512],
                            ps, rdb[:, ih * 512:(ih + 1) * 512],
                        )

                # output projection + bias; y += result (accumulate DMA)
                for co in range(CCH):
                    for ih in range(NIH):
                        ps = pp.tile([128, 512], F32, name=f"ps_y{co}{ih}_b{b}",
                                     tag="mm")
                        for ci in range(CCH):
                            nc.tensor.matmul(
                                ps,
                                wts[("woT", ci)][:, co * 128:(co + 1) * 128],
                                on_t[ci][:, ih * 512:(ih + 1) * 512],
                                start=(ci == 0), stop=(ci == CCH - 1),
                            )
                        # ysb reuses the x slots (x is dead after groupnorm)
                        ysb = dp.tile([128, 512], F32, name=f"ysb{co}{ih}_b{b}",
                                      tag=f"x{co}", bufs=2)
                        nc.vector.tensor_scalar_add(out=ysb, in0=ps, scalar1=bo_pl[co])
                        acc = nc.gpsimd.dma_start(
                            out=y_d[b, co * 128:(co + 1) * 128,
                                    ih * 512:(ih + 1) * 512],
                            in_=ysb, accum_op=ALU.add,
                        )
                        add_dep_helper(acc.ins, copy_insts[(b, co)].ins,
                                       sync=True,
                                       reason="residual base before accumulate")

    nc.finalize()
    return nc


_PROGRAM = None


def _program():
    global _PROGRAM
    if _PROGRAM is None:
        _PROGRAM = _build_program()
    return _PROGRAM


def _round_f32r(a: np.ndarray) -> np.ndarray:
    """Round fp32 to fp32r (11-bit mantissa) round-to-nearest-even."""
    u = np.ascontiguousarray(a, dtype=np.float32).view(np.uint32)
    low = u & np.uint32(0x00000FFF)
    base = u & np.uint32(0xFFFFF000)
    lsb = (u >> np.uint32(12)) & np.uint32(1)
    round_up = (low > 0x800) | ((low == 0x800) & (lsb == 1))
    return (base + (round_up.astype(np.uint32) << np.uint32(12))).view(np.float32)


def kernel(hidden_states, norm_gamma, norm_beta, wq, bq, wk, bk, wv, bv, wo, bo):
    nc = _program()

    x = np.ascontiguousarray(hidden_states, dtype=np.float32).reshape(B, C, N)
    shared = {
        "wqT": _round_f32r(np.ascontiguousarray(np.asarray(wq, np.float32).T)),
        "wkT": _round_f32r(np.ascontiguousarray(np.asarray(wk, np.float32).T)),
        "wvT": _round_f32r(np.ascontiguousarray(np.asarray(wv, np.float32).T)),
        "woT": _round_f32r(np.ascontiguousarray(np.asarray(wo, np.float32).T)),
        "bq": np.ascontiguousarray(bq, np.float32),
        "bk": np.ascontiguousarray(bk, np.float32),
        "bo": np.ascontiguousarray(
            np.asarray(bo, np.float32)
            + np.asarray(wo, np.float32) @ np.asarray(bv, np.float32)
        ),
        "gamma": np.ascontiguousarray(norm_gamma, np.float32),
        "wsel": np.kron(np.eye(128 // GPC, dtype=np.float32),
                        np.full((GPC, GPC), 1.0 / GPC, np.float32)),
        "beta": np.ascontiguousarray(norm_beta, np.float32),
    }
    in_maps = [
        {"x": np.ascontiguousarray(x[c * NB:(c + 1) * NB]), **shared}
        for c in range(NCORES)
    ]
    res = bass_utils.run_bass_kernel_spmd(nc, in_maps, core_ids=list(range(NCORES)))
    out = np.concatenate([res.results[c]["y"] for c in range(NCORES)], axis=0)
    return np.ascontiguousarray(out.reshape(B, C, HH, WW), dtype=np.float32)
